# revision 1
# baseline (speedup 1.0000x reference)
"""ArchetypalNeuralMemory kernel.

Self-contained implementation of the chunked fast-weight memory module
(B=4, S=4096, D=512, CHUNK=64, DEPTH=2).  The per-chunk scan is
inherently serial (each chunk's gradient step feeds the next chunk's
forward), and on this 8-core chip per-step collectives measured ~21us
each (64 steps -> >1.3ms in collective latency alone), so the scan is
computed replicated-dense here; projections are batched matmuls.

kernel(**inputs) takes the FULL unsharded inputs and returns the FULL
output, matching reference.reference().
"""

import numpy as np

DIM = 512
CHUNK = 64
DEPTH = 2
LR = 0.1
MOM = 0.9
DEC = 0.01
EPS_RMS = 1.1920929e-07
EPS_L2 = 1e-12


def _sigmoid(x):
    out = np.empty_like(x)
    np.negative(x, out=out)
    np.exp(out, out=out)
    out += 1.0
    np.reciprocal(out, out=out)
    return out


def _silu(x):
    return x * _sigmoid(x)


def _rmsnorm(x, g):
    ms = np.mean(x * x, axis=-1, keepdims=True, dtype=np.float32)
    return x * (1.0 / np.sqrt(ms + EPS_RMS)) * g


def _l2norm(x):
    n = np.sqrt(np.sum(x * x, axis=-1, keepdims=True, dtype=np.float32))
    return x / np.maximum(n, EPS_L2)


def kernel(x, M, mem_W, Wk, Wv, Wq, Wout, Wgd, bgd, Wgl, bgl, Wgm, bgm, gs, gr):
    x = np.asarray(x, np.float32)
    M = np.asarray(M, np.float32)
    B, S, D = x.shape
    pad = (CHUNK - S % CHUNK) % CHUNK
    if pad > 0:
        x = np.concatenate([x, np.zeros((B, pad, D), np.float32)], axis=1)
    Sp = x.shape[1]
    n = Sp // CHUNK

    # gates: chunk means -> sigmoid -> feature mean -> batch mean
    cm = x.reshape(B, n, CHUNK, D).mean(axis=2, dtype=np.float32)  # [B,n,D]
    alpha = (_sigmoid(cm @ Wgd.T + bgd).mean(-1, dtype=np.float32) * DEC).mean(
        0, dtype=np.float32
    )
    theta = (_sigmoid(cm @ Wgl.T + bgl).mean(-1, dtype=np.float32) * LR).mean(
        0, dtype=np.float32
    )
    eta = (_sigmoid(cm @ Wgm.T + bgm).mean(-1, dtype=np.float32) * MOM).mean(
        0, dtype=np.float32
    )

    x_store = _rmsnorm(x, gs)
    k = _l2norm(_silu(np.einsum("bsd,bde->bse", x_store, M) @ Wk.T))
    v = _silu(x_store @ Wv.T)
    q = _l2norm(_silu(_rmsnorm(x, gr) @ Wq.T))

    # [n, B*CHUNK, D] chunked
    def chunked(t):
        return t.reshape(B, n, CHUNK, D).transpose(1, 0, 2, 3).reshape(n, B * CHUNK, D)

    qc, kc, vc = chunked(q), chunked(k), chunked(v)

    W0 = mem_W[0].astype(np.float32).copy()
    W1 = mem_W[1].astype(np.float32).copy()
    m0 = np.zeros_like(W0)
    m1 = np.zeros_like(W1)

    R = B * CHUNK  # 256 rows per chunk
    inv_n = np.float32(2.0 / (R * D))
    retrieved = np.empty((n, R, D), np.float32)

    for t in range(n):
        q_t, k_t, v_t = qc[t], kc[t], vc[t]
        a, th, et = alpha[t], theta[t], eta[t]

        # retrieve with pre-update weights
        hq = q_t @ W0.T
        retrieved[t] = _silu(hq) @ W1.T

        # forward for the memory loss
        h1 = k_t @ W0.T
        s1 = _sigmoid(h1)
        a1 = h1 * s1
        y = a1 @ W1.T
        dy = (y - v_t) * inv_n  # dL/dy, grad of mean squared error

        # backward
        g1 = dy.T @ a1                       # dW1
        da1 = dy @ W1
        dh1 = da1 * (s1 * (1.0 + h1 * (1.0 - s1)))  # silu'
        g0 = dh1.T @ k_t                     # dW0

        # momentum + decayed weight update
        m0 = et * m0 - th * g0
        m1 = et * m1 - th * g1
        W0 = (1.0 - a) * W0 + m0
        W1 = (1.0 - a) * W1 + m1

    out = (
        retrieved.reshape(n, B, CHUNK, D)
        .transpose(1, 0, 2, 3)
        .reshape(B, Sp, D)[:, :S]
    )
    return (out @ Wout.T).astype(np.float32)



# revision 18
# speedup vs baseline: 558.0645x; 558.0645x over previous
"""ArchetypalNeuralMemory Trainium2 kernel (8-core Bass/Tile SPMD).

Math (reference.py): chunked fast-weight memory, B=4, S=4096, D=512,
CHUNK=64, DEPTH=2.  The 64-step scan is inherently serial (each chunk's
gradient step feeds the next chunk's forward), so the weight-update scan
runs replicated on all 8 cores; projections (k/v/q, gates), retrieval and
the output projection are sharded 8 ways (cyclic chunk ownership) with two
AllGathers (tiny gate scalars early; k/v after the projections).

Layouts: "R" = tokens-on-partitions [r, d]; "D" = features-on-partitions
[d, r].  Forward matmuls contract features -> need D-layout operands;
gradient outer products contract tokens -> need R-layout operands.  Masters
W0T [d,h], W1T [h,f], m0, m1 are fp32 in SBUF; forward matmuls run fp32r
(full PE rate at moving-dim>=256); the gradient path and retrieval run on
bf16 operands.  Per-step bf16 weight snapshots stream to a DRAM history;
each core retrieves its own chunks from the history (register-offset DMA)
interleaved into the scan.
"""

import sys
from contextlib import ExitStack
import numpy as np

for _p in ("/opt/pypackages", "/opt/trn_rl_repo"):
    if _p not in sys.path:
        sys.path.insert(0, _p)

D = 512
CHUNK = 64
B = 4
R = B * CHUNK          # 256 rows per chunk-step
DT = D // 128          # 4 partition tiles of the feature dim
RT = R // 128          # 2 partition tiles of the row dim
LR, MOM, DEC = 0.1, 0.9, 0.01
EPS_RMS = 1.1920929e-07

F32 = None  # filled after imports in _bass_mods
BF16 = None
F32R = None


def _bass_mods():
    import concourse.bass as bass
    import concourse.mybir as mybir
    import concourse.tile as tile
    from concourse import bacc
    from concourse.masks import make_identity
    global F32, BF16, F32R
    F32 = mybir.dt.float32
    BF16 = mybir.dt.bfloat16
    F32R = mybir.dt.float32r
    return bass, mybir, tile, bacc, make_identity


def _bc_replace_part(bassmod, ap, parts):
    """View `ap` with its partition dim replaced by a 0-stride broadcast."""
    return bassmod.AP(tensor=ap.tensor, offset=ap.offset,
                      ap=[[0, parts]] + list(ap.ap)[1:])


def _bc_prepend_part(bassmod, ap, parts):
    """View `ap` with a new leading 0-stride partition broadcast dim."""
    return bassmod.AP(tensor=ap.tensor, offset=ap.offset,
                      ap=[[0, parts]] + list(ap.ap))


def build_program(n_cores=8, n_steps=64, retrieval=True, use_silu=True):
    """Emit the SPMD program. J = n_steps//n_cores chunks owned per core
    (cyclic: core c owns global chunks {k*n_cores + c})."""
    bass, mybir, tile, bacc, make_identity = _bass_mods()
    AF = mybir.ActivationFunctionType
    OP = mybir.AluOpType
    f32, bf16, f32r = F32, BF16, F32R

    NCres = n_cores
    J = n_steps // n_cores          # local chunks per core
    TPC = J * R                     # tokens per core (k/v/q projections)
    NB = TPC // 512                 # 512-wide column blocks of the token dim
    GR = B * J                      # gate rows on this core

    nc = bacc.Bacc("TRN2", target_bir_lowering=False)

    # ---------------- I/O ----------------
    xsT = nc.declare_dram_parameter("xsT", [D, TPC], f32, isOutput=False)
    Mg = nc.declare_dram_parameter("Mg", [B, D, D], f32, isOutput=False)
    WkT = nc.declare_dram_parameter("WkT", [D, D], f32, isOutput=False)
    WvgT = nc.declare_dram_parameter("WvgT", [D, D], f32, isOutput=False)
    WqgT = nc.declare_dram_parameter("WqgT", [D, D], f32, isOutput=False)
    WoutT = nc.declare_dram_parameter("WoutT", [D, D], f32, isOutput=False)
    WgT = nc.declare_dram_parameter("WgT", [D, 3 * D], f32, isOutput=False)
    bg = nc.declare_dram_parameter("bg", [1, 3 * D], f32, isOutput=False)
    W0T0 = nc.declare_dram_parameter("W0T0", [D, D], f32, isOutput=False)
    W1T0 = nc.declare_dram_parameter("W1T0", [D, D], f32, isOutput=False)
    gmix = nc.declare_dram_parameter("gmix", [GR, J], f32, isOutput=False)
    idxbase = nc.declare_dram_parameter("idxbase", [128, 4], mybir.dt.int32,
                                       isOutput=False)
    out = nc.declare_dram_parameter("out", [J, R, D], f32, isOutput=True)

    # ---------------- internal DRAM ----------------
    ktD_loc = nc.dram_tensor("ktD_loc", [D, TPC], bf16)
    kvR_loc = nc.dram_tensor("kvR_loc", [2, TPC, D], bf16)
    g_loc = nc.dram_tensor("g_loc", [J, 4], f32)
    ktD_ag = nc.dram_tensor("ktD_ag", [n_cores * D, TPC], bf16,
                            addr_space="Shared")
    kvR_ag = nc.dram_tensor("kvR_ag", [2 * n_cores, TPC, D], bf16,
                            addr_space="Shared")
    g_ag = nc.dram_tensor("g_ag", [n_steps, 4], f32, addr_space="Shared")
    w0hist = nc.dram_tensor("w0hist", [n_steps, D, D], bf16)
    w1hist = nc.dram_tensor("w1hist", [n_steps, D, D], bf16)
    scal_dram = nc.dram_tensor("scal_dram", [4, n_steps], f32)
    row_dram = nc.dram_tensor("row_dram", [2, TPC], f32)

    with tile.TileContext(nc, num_cores=n_cores) as tc, ExitStack() as ctx:
        consts = ctx.enter_context(tc.tile_pool(name="consts", bufs=1))
        state = ctx.enter_context(tc.tile_pool(name="state", bufs=1))

        # ---- constants / state init ----
        ident = consts.tile([128, 128], f32)
        make_identity(nc, ident)
        identb = consts.tile([128, 128], bf16)
        nc.gpsimd.tensor_copy(identb, ident)
        ones_col = consts.tile([128, 1], f32)
        nc.vector.memset(ones_col, 1.0)

        _csts = {}

        def cst(val):
            if val not in _csts:
                tt = consts.tile([128, 1], f32, tag=f"cst{len(_csts)}")
                nc.gpsimd.memset(tt, val)
                _csts[val] = tt
            return _csts[val]

        def act_silu(dst, src, pool, tag):
            """dst = silu(src); src may be PSUM."""
            if use_silu:
                nc.scalar.activation(dst, src, AF.Silu)
            else:
                s1 = pool.tile(list(dst.shape), f32, tag=tag)
                nc.scalar.activation(s1, src, AF.Sigmoid)
                nc.vector.tensor_tensor(dst, src, s1, op=OP.mult)

        w0 = state.tile([128, DT, D], f32, tag="w0")
        w1 = state.tile([128, DT, D], f32, tag="w1")
        m0 = state.tile([128, DT, D], f32, tag="m0")
        m1 = state.tile([128, DT, D], f32, tag="m1")
        nc.sync.dma_start(out=w0, in_=W0T0.rearrange("(a p) e -> p a e", p=128))
        nc.sync.dma_start(out=w1, in_=W1T0.rearrange("(a p) e -> p a e", p=128))
        nc.vector.memset(m0, 0.0)
        nc.vector.memset(m1, 0.0)

        woutb = state.tile([128, DT, D], bf16, tag="woutb")

        idxc = consts.tile([128, 4], mybir.dt.int32)
        nc.sync.dma_start(out=idxc, in_=idxbase[:])


        # ================= projections =================
        # SBUF staging: big fp32 [128,DT,TPC] tiles share three 32KB/part
        # slots -- A: xs -> Mg -> k3 ; B: gates-W -> v3 -> kp -> q3 ; C: xst.
        qtD = nc.dram_tensor("qtD", [D, TPC], bf16)
        vbf_dram = nc.dram_tensor("vbf_dram", [D, TPC], bf16)

        with tc.tile_pool(name="proj", bufs=1) as proj, \
             tc.tile_pool(name="projsm", bufs=1) as projsm, \
             tc.tile_pool(name="projw", bufs=1) as projw, \
             tc.tile_pool(name="projx", bufs=2) as projx:
            xs = proj.tile([128, DT, TPC], f32, tag="A")
            nc.sync.dma_start(out=xs, in_=xsT.rearrange("(a p) e -> p a e", p=128))

            # --- gates: chunk sums (free-axis reduce over pos) ---
            wg_sb = proj.tile([128, DT, 3 * D], f32, tag="B")
            nc.sync.dma_start(out=wg_sb, in_=WgT.rearrange("(a p) e -> p a e", p=128))
            bg_bc = projsm.tile([GR, 3 * D], f32, tag="bgbc")
            nc.sync.dma_start(out=bg_bc, in_=_bc_replace_part(bass, bg[:], GR))
            gmix_sb = projsm.tile([GR, J], f32, tag="gmix")
            nc.sync.dma_start(out=gmix_sb, in_=gmix[:])

            gatep_cm = tc.tile_pool(name="gatep", bufs=1, space="PSUM")
            gatep = gatep_cm.__enter__()
            cms = projsm.tile([128, DT, GR], f32, tag="cms")
            xs_v = xs.rearrange("p a (g s) -> p a g s", s=CHUNK)
            for di in range(DT):
                nc.vector.tensor_reduce(
                    out=cms[:, di, :], in_=xs_v[:, di, :, :],
                    axis=mybir.AxisListType.X, op=OP.add,
                )
            zps = gatep.tile([GR, 3, D], f32, tag="zps")
            for nb3 in range(3):
                for di in range(DT):
                    nc.tensor.matmul(
                        zps[:, nb3, :], cms[:, di, :],
                        wg_sb[:, di, nb3 * D:(nb3 + 1) * D],
                        start=(di == 0), stop=(di == DT - 1),
                    )
            zsb = projsm.tile([GR, 3 * D], f32, tag="zsb")
            for nb3 in range(3):
                nc.vector.scalar_tensor_tensor(
                    out=zsb[:, nb3 * D:(nb3 + 1) * D], in0=zps[:, nb3, :],
                    scalar=1.0 / CHUNK, in1=bg_bc[:, nb3 * D:(nb3 + 1) * D],
                    op0=OP.mult, op1=OP.add,
                )
            nc.scalar.activation(zsb, zsb, AF.Sigmoid)
            rs = projsm.tile([GR, 3], f32, tag="rs")
            nc.vector.tensor_reduce(
                out=rs, in_=zsb.rearrange("p (a e) -> p a e", a=3),
                axis=mybir.AxisListType.X, op=OP.add,
            )
            gps = gatep.tile([J, 4], f32, tag="gps")
            nc.tensor.matmul(gps[:, 0:3], gmix_sb, rs, start=True, stop=True)
            gsb = projsm.tile([J, 4], f32, tag="gsb")
            nc.vector.memset(gsb, 0.0)
            for i, const in enumerate((DEC, LR, MOM)):
                nc.vector.tensor_scalar_mul(
                    gsb[:, i:i + 1], gps[:, i:i + 1], const / (4 * D)
                )
            nc.sync.dma_start(out=g_loc[:], in_=gsb)
            if n_cores > 1:
                nc.gpsimd.collective_compute(
                    "AllGather", OP.bypass,
                    replica_groups=[list(range(n_cores))],
                    ins=[g_loc[:]], outs=[g_ag[:]],
                )
            else:
                nc.sync.dma_start(out=g_ag[:], in_=g_loc[:])

            # --- scalar tables: (1-a), eta, s2 = -th*2/(R*D) ---
            gfull = projsm.tile([n_steps, 4], f32, tag="gfull")
            nc.sync.dma_start(out=gfull, in_=g_ag[:])
            packed = projsm.tile([128, 128], f32, tag="packed")
            nc.vector.memset(packed, 0.0)
            nc.scalar.activation(packed[0:n_steps, 0:1], gfull[:, 0:1],
                                 AF.Copy, bias=1.0, scale=-1.0)
            nc.vector.tensor_copy(packed[0:n_steps, 1:2], gfull[:, 2:3])
            nc.vector.tensor_scalar_mul(packed[0:n_steps, 2:3], gfull[:, 1:2],
                                        -2.0 / (R * D))
            tblps = gatep.tile([128, 128], f32, tag="tblps")
            nc.tensor.transpose(tblps, packed, ident)
            tbl_sm = projsm.tile([4, n_steps], f32, tag="tblsm")
            nc.scalar.activation(tbl_sm, tblps[0:4, 0:n_steps], AF.Copy)
            nc.sync.dma_start(out=scal_dram[0:4, :], in_=tbl_sm)
            TBL = state.tile([128, 4, n_steps], f32, tag="tbl")
            nc.sync.dma_start(out=TBL,
                              in_=_bc_prepend_part(bass, scal_dram[:], 128))
            gatep_cm.__exit__(None, None, None)
            projp_cm = tc.tile_pool(name="projp", bufs=2, space="PSUM")
            projp = projp_cm.__enter__()

            # --- rmsnorm (no gain; gains folded into Mg/WvgT/WqgT) ---
            for nb in range(NB):
                msp = projp.tile([1, 512], f32, tag="msp")
                for di in range(DT):
                    sqd = projsm.tile([128, 512], f32, tag="sqd")
                    nc.scalar.activation(sqd, xs[:, di, nb * 512:(nb + 1) * 512],
                                         AF.Square)
                    nc.tensor.matmul(msp, ones_col, sqd,
                                     start=(di == 0), stop=(di == DT - 1))
                srw = projsm.tile([1, 512], f32, tag="sqrow")
                nc.scalar.activation(srw, msp, AF.Sqrt,
                                     bias=cst(EPS_RMS)[0:1], scale=1.0 / D)
                rrw = projsm.tile([1, 512], f32, tag="rrow")
                nc.vector.reciprocal(rrw, srw)
                nc.sync.dma_start(out=row_dram[0:1, nb * 512:(nb + 1) * 512],
                                  in_=rrw)
            rbc0 = projsm.tile([128, TPC], f32, tag="rowbc")
            nc.sync.dma_start(out=rbc0,
                              in_=_bc_replace_part(bass, row_dram[0:1, :], 128))
            xst = proj.tile([128, DT, TPC], f32, tag="C")
            for di in range(DT):
                nc.vector.tensor_tensor(xst[:, di, :], xs[:, di, :], rbc0,
                                        op=OP.mult)

            def proj_silu(w_sb, src, dst):
                """dst[128,DT,TPC] = silu(w_sb.T @ src), D-layout."""
                for fj in range(DT):
                    for nb in range(NB):
                        pp = projp.tile([128, 512], f32, tag="pp")
                        for di in range(DT):
                            nc.tensor.matmul(
                                pp, w_sb[:, di, fj * 128:(fj + 1) * 128],
                                src[:, di, nb * 512:(nb + 1) * 512],
                                start=(di == 0), stop=(di == DT - 1),
                            )
                        act_silu(dst[:, fj, nb * 512:(nb + 1) * 512], pp,
                                 projsm, "psilu")

            def l2_scale_inplace(src):
                """src *= 1/max(||src||_col, eps) over partition(feature) axis."""
                for nb in range(NB):
                    nps = projp.tile([1, 512], f32, tag="msp")
                    for di in range(DT):
                        sqd = projsm.tile([128, 512], f32, tag="sqd")
                        nc.scalar.activation(sqd,
                                             src[:, di, nb * 512:(nb + 1) * 512],
                                             AF.Square)
                        nc.tensor.matmul(nps, ones_col, sqd,
                                         start=(di == 0), stop=(di == DT - 1))
                    srw = projsm.tile([1, 512], f32, tag="sqrow")
                    nc.scalar.activation(srw, nps, AF.Sqrt, bias=cst(1e-30)[0:1])
                    rrw = projsm.tile([1, 512], f32, tag="rrow")
                    nc.vector.reciprocal(rrw, srw)
                    nc.sync.dma_start(out=row_dram[1:2, nb * 512:(nb + 1) * 512],
                                      in_=rrw)
                rbc = projsm.tile([128, TPC], f32, tag="rowbc")
                nc.sync.dma_start(out=rbc,
                                  in_=_bc_replace_part(bass, row_dram[1:2, :], 128))
                for di in range(DT):
                    nc.vector.tensor_tensor(src[:, di, :], src[:, di, :], rbc,
                                            op=OP.mult)

            def cast_out(src, dram_bf):
                """bf16-cast src [128,DT,TPC] to DRAM [D,TPC] via 8KB pieces."""
                for di in range(DT):
                    bfd = projx.tile([128, TPC], bf16, tag="bfd")
                    nc.gpsimd.tensor_copy(bfd, src[:, di, :])
                    nc.sync.dma_start(
                        out=dram_bf[di * 128:(di + 1) * 128, :], in_=bfd)

            def transpose_to_R(dram_bf, loc):
                """DMA-transpose DRAM [D,TPC] bf16 -> loc [TPC,D] bf16."""
                loc_v = loc.rearrange("(t p) e -> p t e", p=128)
                for tq in range(TPC // 512):
                    rq = projx.tile([128, 4, D], bf16, tag="Rq")
                    for i in range(4):
                        ti = tq * 4 + i
                        nc.sync.dma_start_transpose(
                            out=rq[:, i, :],
                            in_=dram_bf[:, ti * 128:(ti + 1) * 128])
                    nc.sync.dma_start(out=loc_v[:, tq * 4:(tq + 1) * 4, :], in_=rq)

            wout_f = projw.tile([128, DT, D], f32, tag="wproj")
            nc.sync.dma_start(out=wout_f,
                              in_=WoutT.rearrange("(a p) e -> p a e", p=128))
            nc.gpsimd.tensor_copy(woutb, wout_f)

            # --- v ---
            wv_sb = projw.tile([128, DT, D], f32, tag="wproj")
            nc.sync.dma_start(out=wv_sb,
                              in_=WvgT.rearrange("(a p) e -> p a e", p=128))
            v3 = proj.tile([128, DT, TPC], f32, tag="B")
            proj_silu(wv_sb, xst, v3)
            cast_out(v3, vbf_dram)
            transpose_to_R(vbf_dram, kvR_loc[1])

            # --- k ---
            mg_sb = proj.tile([128, B * DT, D], f32, tag="A")
            nc.sync.dma_start(out=mg_sb,
                              in_=Mg.rearrange("b (a p) e -> p (b a) e", p=128))
            xst_v = xst.rearrange("p a (j b s) -> p a j b s", b=B, s=CHUNK)
            kp = proj.tile([128, DT, TPC], f32, tag="B")
            kp_v = kp.rearrange("p a (j b s) -> p a j b s", b=B, s=CHUNK)
            PB = TPC // B
            for b in range(B):
                for ej in range(DT):
                    ppb = projp.tile([128, PB], f32, tag="ppb")
                    for di in range(DT):
                        nc.tensor.matmul(
                            ppb, mg_sb[:, b * DT + di, ej * 128:(ej + 1) * 128],
                            xst_v[:, di, :, b, :],
                            start=(di == 0), stop=(di == DT - 1),
                        )
                    nc.scalar.activation(kp_v[:, ej, :, b, :], ppb, AF.Copy)
            wk_sb = projw.tile([128, DT, D], f32, tag="wproj")
            nc.sync.dma_start(out=wk_sb,
                              in_=WkT.rearrange("(a p) e -> p a e", p=128))
            k3 = proj.tile([128, DT, TPC], f32, tag="A")
            proj_silu(wk_sb, kp, k3)
            l2_scale_inplace(k3)
            cast_out(k3, ktD_loc)
            transpose_to_R(ktD_loc, kvR_loc[0])

            # --- q (bf16 D-layout to DRAM; streamed at retrieval) ---
            wq_sb = projw.tile([128, DT, D], f32, tag="wproj")
            nc.sync.dma_start(out=wq_sb,
                              in_=WqgT.rearrange("(a p) e -> p a e", p=128))
            q3 = proj.tile([128, DT, TPC], f32, tag="B")
            proj_silu(wq_sb, xst, q3)
            l2_scale_inplace(q3)
            cast_out(q3, qtD)

            if n_cores > 1:
                nc.gpsimd.collective_compute(
                    "AllGather", OP.bypass,
                    replica_groups=[list(range(n_cores))],
                    ins=[ktD_loc[:]], outs=[ktD_ag[:]],
                )
                nc.gpsimd.collective_compute(
                    "AllGather", OP.bypass,
                    replica_groups=[list(range(n_cores))],
                    ins=[kvR_loc[:]], outs=[kvR_ag[:]],
                )
            else:
                nc.sync.dma_start(out=ktD_ag[:], in_=ktD_loc[:])
                nc.sync.dma_start(out=kvR_ag[:], in_=kvR_loc[:])
            projp_cm.__exit__(None, None, None)

        # ================= scan =================
        ktd_ag_v = ktD_ag.rearrange("(c a p) e -> c p a e", c=n_cores, p=128)
        kvr_ag_v = kvR_ag.rearrange("(c x) (j r p) e -> c x j p r e",
                                    x=2, j=J, p=128)

        sp = ctx.enter_context(tc.tile_pool(name="scan", bufs=2))
        spst = ctx.enter_context(tc.tile_pool(name="scanstream", bufs=3))
        ppf = ctx.enter_context(tc.tile_pool(name="ppf", bufs=1, space="PSUM"))
        ppg = ctx.enter_context(tc.tile_pool(name="ppg", bufs=4, space="PSUM"))
        rp = ctx.enter_context(tc.tile_pool(name="ret", bufs=2))
        rpp = ctx.enter_context(tc.tile_pool(name="retp", bufs=1, space="PSUM"))

        def retrieve(jj):
            """Retrieve local chunk jj (global chunk jj*n_cores + cid) from the
            bf16 weight history, apply Wout, write out[jj]."""
            idx = rp.tile([128, 4], mybir.dt.int32, tag="idx")
            nc.vector.tensor_scalar_add(idx, idxc, jj * n_cores * D)
            w0h = rp.tile([128, DT, D], bf16, tag="w0h")
            w1h = rp.tile([128, DT, D], bf16, tag="w1h")
            nvalid = (jj + 1) * n_cores * D
            for a in range(DT):
                nc.gpsimd.indirect_dma_start(
                    out=w0h[:, a, :], out_offset=None,
                    in_=w0hist.rearrange("t r e -> (t r) e")[0:nvalid, :],
                    in_offset=bass.IndirectOffsetOnAxis(ap=idx[:, a:a + 1], axis=0),
                )
                nc.gpsimd.indirect_dma_start(
                    out=w1h[:, a, :], out_offset=None,
                    in_=w1hist.rearrange("t r e -> (t r) e")[0:nvalid, :],
                    in_offset=bass.IndirectOffsetOnAxis(ap=idx[:, a:a + 1], axis=0),
                )
            qts = rp.tile([128, DT, R], bf16, tag="qts")
            nc.sync.dma_start(
                out=qts,
                in_=qtD[:, jj * R:(jj + 1) * R].rearrange("(a p) r -> p a r", p=128))
            hr = rpp.tile([128, RT, D], f32, tag="ret")
            for rj in range(RT):
                for di in range(DT):
                    nc.tensor.matmul(
                        hr[:, rj, :],
                        qts[:, di, rj * 128:(rj + 1) * 128],
                        w0h[:, di, :], start=(di == 0), stop=(di == DT - 1),
                    )
            ar = rp.tile([128, RT, D], bf16, tag="ar")
            for rj in range(RT):
                act_silu(ar[:, rj, :], hr[:, rj, :], rp, "rsilu")
            arT = rpp.tile([128, DT, R], bf16, tag="ret")
            for hj in range(DT):
                for rj in range(RT):
                    nc.tensor.transpose(
                        arT[:, hj, rj * 128:(rj + 1) * 128],
                        ar[:, rj, hj * 128:(hj + 1) * 128], identb,
                    )
            arTs = rp.tile([128, DT, R], bf16, tag="arTs")
            nc.scalar.activation(arTs, arT, AF.Copy)
            orp = rpp.tile([128, RT, D], f32, tag="ret")
            for rj in range(RT):
                for hj in range(DT):
                    nc.tensor.matmul(
                        orp[:, rj, :], arTs[:, hj, rj * 128:(rj + 1) * 128],
                        w1h[:, hj, :], start=(hj == 0), stop=(hj == DT - 1),
                    )
            ors = rp.tile([128, RT, D], bf16, tag="ors")
            for rj in range(RT):
                nc.scalar.activation(ors[:, rj, :], orp[:, rj, :], AF.Copy)
            orT = rpp.tile([128, DT, R], bf16, tag="ret")
            for fi in range(DT):
                for rj in range(RT):
                    nc.tensor.transpose(
                        orT[:, fi, rj * 128:(rj + 1) * 128],
                        ors[:, rj, fi * 128:(fi + 1) * 128], identb,
                    )
            orTs = rp.tile([128, DT, R], bf16, tag="orTs")
            nc.scalar.activation(orTs, orT, AF.Copy)
            fin = rpp.tile([128, RT, D], f32, tag="ret")
            for rj in range(RT):
                for fi in range(DT):
                    nc.tensor.matmul(
                        fin[:, rj, :], orTs[:, fi, rj * 128:(rj + 1) * 128],
                        woutb[:, fi, :], start=(fi == 0), stop=(fi == DT - 1),
                    )
            fsb = rp.tile([128, RT, D], f32, tag="fsb")
            for rj in range(RT):
                nc.scalar.activation(fsb[:, rj, :], fin[:, rj, :], AF.Copy)
            nc.sync.dma_start(
                out=out[jj].rearrange("(a p) e -> p a e", p=128), in_=fsb
            )

        for t in range(n_steps):
            rank, j = t % n_cores, t // n_cores
            onema = TBL[:, 0, t:t + 1]
            eta_s = TBL[:, 1, t:t + 1]
            s2_s = TBL[:, 2, t:t + 1]

            kt = spst.tile([128, DT, R], bf16, tag="kt")
            nc.sync.dma_start(out=kt, in_=ktd_ag_v[rank, :, :, j * R:(j + 1) * R])
            kr = spst.tile([128, RT, D], bf16, tag="kr")
            nc.sync.dma_start(out=kr, in_=kvr_ag_v[rank, 0, j])
            vr = spst.tile([128, RT, D], bf16, tag="vr")
            nc.sync.dma_start(out=vr, in_=kvr_ag_v[rank, 1, j])

            # bf16 weight snapshots -> history (also feeds u1fh transpose)
            w0hb = sp.tile([128, DT, D], bf16, tag="w0hb")
            w1hb = sp.tile([128, DT, D], bf16, tag="w1hb")
            for di in range(DT):
                nc.gpsimd.tensor_copy(w0hb[:, di, :], w0[:, di, :])
                nc.gpsimd.tensor_copy(w1hb[:, di, :], w1[:, di, :])
            nc.sync.dma_start(
                out=w0hist[t].rearrange("(a p) e -> p a e", p=128), in_=w0hb)
            nc.sync.dma_start(
                out=w1hist[t].rearrange("(a p) e -> p a e", p=128), in_=w1hb)
            u1fh = sp.tile([128, DT, D], bf16, tag="u1fh")
            for fi in range(DT):
                for hj in range(DT):
                    nc.sync.dma_start_transpose(
                        out=u1fh[:, fi, hj * 128:(hj + 1) * 128],
                        in_=w1hist[t, hj * 128:(hj + 1) * 128,
                                   fi * 128:(fi + 1) * 128],
                    )

            # forward: h = k-tilde.T @ W0T   (fp32r)
            hp = ppf.tile([128, RT, D], f32, tag="fwd")
            for rj in range(RT):
                for di in range(DT):
                    nc.tensor.matmul(
                        hp[:, rj, :],
                        kt[:, di, rj * 128:(rj + 1) * 128],
                        w0hb[:, di, :],
                        start=(di == 0), stop=(di == DT - 1),
                    )
            a1 = sp.tile([128, RT, D], bf16, tag="a1")
            deriv = sp.tile([128, RT, D], bf16, tag="deriv")
            if use_silu:
                for rj in range(RT):
                    nc.scalar.activation(a1[:, rj, :], hp[:, rj, :], AF.Silu)
                    nc.scalar.activation(deriv[:, rj, :], hp[:, rj, :],
                                         AF.Derivative_silu)
            else:
                s1t = sp.tile([128, RT, D], bf16, tag="s1t")
                u_t = sp.tile([128, RT, D], bf16, tag="u_t")
                for rj in range(RT):
                    nc.scalar.activation(s1t[:, rj, :], hp[:, rj, :], AF.Sigmoid)
                    nc.vector.tensor_tensor(a1[:, rj, :], hp[:, rj, :],
                                            s1t[:, rj, :], op=OP.mult)
                    # deriv = s1 + a1*(1 - s1) = s1 + a1 - a1*s1
                    nc.vector.tensor_tensor(u_t[:, rj, :], a1[:, rj, :],
                                            s1t[:, rj, :], op=OP.mult)
                    nc.vector.tensor_tensor(deriv[:, rj, :], s1t[:, rj, :],
                                            a1[:, rj, :], op=OP.add)
                    nc.vector.tensor_tensor(deriv[:, rj, :], deriv[:, rj, :],
                                            u_t[:, rj, :], op=OP.subtract)
            a1s = sp.tile([128, RT, D], bf16, tag="a1s")
            for rj in range(RT):
                nc.gpsimd.tensor_scalar(a1s[:, rj, :], a1[:, rj, :], s2_s, None,
                                        op0=OP.mult)
            a1T = ppf.tile([128, DT, R], bf16, tag="fwd")
            for hj in range(DT):
                for rj in range(RT):
                    nc.tensor.transpose(
                        a1T[:, hj, rj * 128:(rj + 1) * 128],
                        a1[:, rj, hj * 128:(hj + 1) * 128],
                        identb,
                    )
            a1Ts = sp.tile([128, DT, R], bf16, tag="a1Ts")
            nc.scalar.activation(a1Ts, a1T, AF.Copy)

            yp = ppf.tile([128, RT, D], f32, tag="fwd")
            for rj in range(RT):
                for hj in range(DT):
                    nc.tensor.matmul(
                        yp[:, rj, :],
                        a1Ts[:, hj, rj * 128:(rj + 1) * 128],
                        w1hb[:, hj, :],
                        start=(hj == 0), stop=(hj == DT - 1),
                    )
            dy = sp.tile([128, RT, D], bf16, tag="dy")
            for rj in range(RT):
                nc.vector.tensor_tensor(dy[:, rj, :], yp[:, rj, :], vr[:, rj, :],
                                        op=OP.subtract)
            dyT = ppf.tile([128, DT, R], bf16, tag="fwd")
            for fi in range(DT):
                for rj in range(RT):
                    nc.tensor.transpose(
                        dyT[:, fi, rj * 128:(rj + 1) * 128],
                        dy[:, rj, fi * 128:(fi + 1) * 128], identb,
                    )
            dyTs = sp.tile([128, DT, R], bf16, tag="dyTs")
            nc.scalar.activation(dyTs, dyT, AF.Copy)

            dap = ppf.tile([128, RT, D], f32, tag="fwd")
            for rj in range(RT):
                for fi in range(DT):
                    nc.tensor.matmul(
                        dap[:, rj, :], dyTs[:, fi, rj * 128:(rj + 1) * 128],
                        u1fh[:, fi, :], start=(fi == 0), stop=(fi == DT - 1),
                    )
            dh = sp.tile([128, RT, D], bf16, tag="dh")
            for rj in range(RT):
                nc.vector.scalar_tensor_tensor(
                    out=dh[:, rj, :], in0=dap[:, rj, :], scalar=s2_s,
                    in1=deriv[:, rj, :], op0=OP.mult, op1=OP.mult,
                )

            for dj in range(DT):
                g0t = ppg.tile([128, D], f32, tag="g")
                for ri in range(RT):
                    nc.tensor.matmul(
                        g0t, kr[:, ri, dj * 128:(dj + 1) * 128],
                        dh[:, ri, :], start=(ri == 0), stop=(ri == RT - 1),
                    )
                nc.vector.scalar_tensor_tensor(
                    out=m0[:, dj, :], in0=m0[:, dj, :], scalar=eta_s,
                    in1=g0t, op0=OP.mult, op1=OP.add,
                )
                nc.vector.scalar_tensor_tensor(
                    out=w0[:, dj, :], in0=w0[:, dj, :], scalar=onema,
                    in1=m0[:, dj, :], op0=OP.mult, op1=OP.add,
                )
            for hj in range(DT):
                g1t = ppg.tile([128, D], f32, tag="g")
                for ri in range(RT):
                    nc.tensor.matmul(
                        g1t, a1s[:, ri, hj * 128:(hj + 1) * 128],
                        dy[:, ri, :], start=(ri == 0), stop=(ri == RT - 1),
                    )
                nc.vector.scalar_tensor_tensor(
                    out=m1[:, hj, :], in0=m1[:, hj, :], scalar=eta_s,
                    in1=g1t, op0=OP.mult, op1=OP.add,
                )
                nc.vector.scalar_tensor_tensor(
                    out=w1[:, hj, :], in0=w1[:, hj, :], scalar=onema,
                    in1=m1[:, hj, :], op0=OP.mult, op1=OP.add,
                )

            if retrieval and (t % n_cores) == n_cores - 1:
                retrieve(t // n_cores)

    nc.compile()
    return nc


# ---------------------------------------------------------------------------
# host side
# ---------------------------------------------------------------------------

_PROGRAM_CACHE = {}


def _get_program(n_cores, n_steps):
    key = (n_cores, n_steps)
    if key not in _PROGRAM_CACHE:
        _PROGRAM_CACHE[key] = build_program(n_cores=n_cores, n_steps=n_steps)
    return _PROGRAM_CACHE[key]


def host_inputs(x, M, mem_W, Wk, Wv, Wq, Wout, Wgd, bgd, Wgl, bgl, Wgm, bgm,
                gs, gr, n_cores=8):
    """Build the per-core in_maps (pure slicing/transposition/weight prep)."""
    x = np.ascontiguousarray(np.asarray(x, np.float32))
    Bx, S, Dx = x.shape
    n = S // CHUNK
    J = n // n_cores
    TPC = J * R
    GR = B * J

    Mg = (np.asarray(M, np.float32) * np.asarray(gs, np.float32)[None, :, None])
    WkT = np.ascontiguousarray(np.asarray(Wk, np.float32).T)
    WvgT = np.ascontiguousarray((np.asarray(Wv, np.float32)
                                 * np.asarray(gs, np.float32)[None, :]).T)
    WqgT = np.ascontiguousarray((np.asarray(Wq, np.float32)
                                 * np.asarray(gr, np.float32)[None, :]).T)
    WoutT = np.ascontiguousarray(np.asarray(Wout, np.float32).T)
    WgT = np.ascontiguousarray(
        np.concatenate([np.asarray(Wgd, np.float32).T,
                        np.asarray(Wgl, np.float32).T,
                        np.asarray(Wgm, np.float32).T], axis=1))
    bgc = np.concatenate([np.asarray(bgd, np.float32),
                          np.asarray(bgl, np.float32),
                          np.asarray(bgm, np.float32)])[None, :]
    W0T0 = np.ascontiguousarray(np.asarray(mem_W[0], np.float32).T)
    W1T0 = np.ascontiguousarray(np.asarray(mem_W[1], np.float32).T)
    gmix = np.zeros((GR, J), np.float32)
    for jj in range(J):
        gmix[jj * B:(jj + 1) * B, jj] = 1.0
    # x chunked [n, R, D] with rows (b, pos)
    xc = x.reshape(B, n, CHUNK, Dx).transpose(1, 0, 2, 3).reshape(n, R, Dx)

    in_maps = []
    for c in range(n_cores):
        chunks = [k * n_cores + c for k in range(J)]
        xs = xc[chunks].reshape(TPC, Dx)          # rows (jj, b, pos)
        xsT = np.ascontiguousarray(xs.T)
        idxbase = (np.int32(c) * np.int32(D)
                   + np.arange(4, dtype=np.int32)[None, :] * 128
                   + np.arange(128, dtype=np.int32)[:, None]).astype(np.int32)
        in_maps.append({
            "xsT": xsT, "Mg": Mg, "WkT": WkT, "WvgT": WvgT, "WqgT": WqgT,
            "WoutT": WoutT, "WgT": WgT, "bg": bgc, "W0T0": W0T0, "W1T0": W1T0,
            "gmix": gmix, "idxbase": idxbase,
        })
    return in_maps


def assemble_output(results, S=4096, n_cores=8):
    n = S // CHUNK
    J = n // n_cores
    outc = np.empty((n, R, D), np.float32)
    for c in range(n_cores):
        o = results[c]["out"]                     # [J, R, D]
        for jj in range(J):
            outc[jj * n_cores + c] = o[jj]
    return (outc.reshape(n, B, CHUNK, D).transpose(1, 0, 2, 3)
            .reshape(B, n * CHUNK, D))


def kernel(x, M, mem_W, Wk, Wv, Wq, Wout, Wgd, bgd, Wgl, bgl, Wgm, bgm, gs, gr):
    from concourse.bass_utils import run_bass_kernel_spmd
    n_cores = 8
    nc = _get_program(n_cores, 64)
    in_maps = host_inputs(x, M, mem_W, Wk, Wv, Wq, Wout, Wgd, bgd, Wgl, bgl,
                          Wgm, bgm, gs, gr, n_cores=n_cores)
    res = run_bass_kernel_spmd(nc, in_maps, core_ids=list(range(n_cores)))
    return assemble_output(res.results, S=x.shape[1], n_cores=n_cores)
